# revision 1
# baseline (speedup 1.0000x reference)
"""GNN scatter-mean (SimpleConv mean + self-loop, threshold col 0) on 8 trn2 cores.

Design (per NeuronCore c of 8):
  - owns nodes [12500c, 12500(c+1)); edges bucketed by dst owner (host).
  - only column 0 of x matters: out[i] = (sum_{j->i} s[j] + s[i] > 0), s = x[:,0]
    (degree >= 1 so the mean's sign equals the sum's sign).
  - 8 Q7-core groups by src chunk (12500 each); per group a replicated SBUF
    table of its s-chunk (+ zero slot); ap_gather fetches s[src] per edge in
    dst-sorted order into 16 row-streams per group (host-balanced rows of
    nodes, identical row ranges across groups).
  - custom DVE cumsum over the [128, CROW] canvas; local_scatter extracts
    P at run ends into per-node packed slots; shifted subtract gives per
    (group,row) node partial sums; a [128x16] 0/1 matmul sums the 8 groups;
    add s_own, threshold > 0.
"""
import numpy as np

import concourse.bass as bass
import concourse.bacc as bacc
import concourse.mybir as mybir
import concourse.tile as tile

# ---------------------------------------------------------------- constants
N_NODES = 100000
N_CORES = 8
NN = N_NODES // N_CORES      # 12500 nodes per core
K = 8                        # src-chunk groups (one per Q7 core)
RR = 16                      # rows per group (one per partition in group)
CROW = 3312                  # stream slots per (group,row)
GC = 8                       # gather calls (2 rows per call)
IDXC = 2 * CROW // 16        # per-core idx cols per call (828B slices, 4B-aligned)
NSLOT = 960                  # packed per-node slots per row (8 chunks x 120)
MCH = 120                    # matmul chunk (psum partitions)
ZSLOT = NN                   # table slot holding 0.0
TBL = 12800                  # table free size (2KB-mult padded)
F32 = mybir.dt.float32
I16 = mybir.dt.int16

_CUMSUM_OP = None


def _register_cumsum():
    global _CUMSUM_OP
    if _CUMSUM_OP is not None:
        return _CUMSUM_OP
    import concourse.dve_ops as dve_ops
    from concourse.dve_ops import DveOp, OPS, CUSTOM_DVE_SPECS, _SUB_OPCODE_FOR_NAME
    from concourse.dve_spec import Spec, Src0, scan, AluOp, lower
    from concourse.dve_uop import DveOpSpec

    name = "CUMSUM_ANT_GNN"
    if name in _SUB_OPCODE_FOR_NAME:
        _CUMSUM_OP = next(o for o in OPS if o.name == name)
        return _CUMSUM_OP
    spec = Spec(
        body=scan(AluOp.ADD, Src0),
        reference=lambda in0, in1, s0, s1, imm2: np.cumsum(
            np.asarray(in0, np.float32), axis=-1, dtype=np.float32
        ),
    )
    opcode = 1 + len(OPS)
    shas = {}
    for ver in ("v3", "v4"):
        s = DveOpSpec(name=name, opcode=opcode, uops=lower(spec, ver=ver), rd1_en=False)
        shas[ver] = s.sha(ver)
    op = DveOp(name, spec, subdim=False, uops_sha=shas)
    OPS.append(op)
    CUSTOM_DVE_SPECS[name] = spec
    _SUB_OPCODE_FOR_NAME[name] = opcode
    _CUMSUM_OP = op
    return op


# ---------------------------------------------------------------- device IR
def build_nc(num_devices=N_CORES, repeat=1, debug_taps=False, ablate=()):
    cum_op = _register_cumsum()
    nc = bacc.Bacc("TRN2", target_bir_lowering=False, debug=False,
                   num_devices=num_devices)
    s_chunks = nc.dram_tensor("s_chunks", [K, TBL], F32, kind="ExternalInput")
    gidx = nc.dram_tensor("gidx", [128, GC * IDXC], I16, kind="ExternalInput")
    bidx = nc.dram_tensor("bidx", [128, 2 * CROW], I16, kind="ExternalInput")
    sown = nc.dram_tensor("sown", [MCH, 128], F32, kind="ExternalInput")
    selm = nc.dram_tensor("selm", [128, 16], F32, kind="ExternalInput")
    perm = nc.dram_tensor("perm", [128, 16 * 128], F32, kind="ExternalInput")
    y = nc.dram_tensor("y", [MCH, 128], F32, kind="ExternalOutput")
    taps = {}
    if debug_taps:
        for tn, shp in (("tap_canvas", [128, CROW]), ("tap_pfx", [128, CROW]),
                        ("tap_packed", [128, NSLOT]), ("tap_dif", [128, NSLOT]),
                        ("tap_gout0", [128, CROW])):
            taps[tn] = nc.dram_tensor(tn, shp, F32, kind="ExternalOutput")

    with tile.TileContext(nc) as tc:
        with (
            tc.tile_pool(name="const", bufs=1) as cpool,
            tc.tile_pool(name="gout", bufs=1) as gpool,
            tc.tile_pool(name="work", bufs=1) as wpool,
            tc.tile_pool(name="psum", bufs=1, space="PSUM") as ppool,
        ):
            table = cpool.tile([128, TBL], F32, tag="table")
            # replicate chunk k into partitions 16k..16k+15 with one DMA
            for kk in range(K):
                nc.sync.dma_start(
                    out=table[16 * kk:16 * (kk + 1), :],
                    in_=s_chunks.ap()[kk:kk + 1, :].to_broadcast([16, TBL]))

            gidx_t = cpool.tile([128, 4096], I16, tag="gidx")
            nc.sync.dma_start(out=gidx_t[:, :GC * IDXC], in_=gidx.ap())
            bidx_t = cpool.tile([128, 8192], I16, tag="bidx")
            nc.sync.dma_start(out=bidx_t[:, :2 * CROW], in_=bidx.ap())
            sown_t = cpool.tile([MCH, 512], F32, tag="sown")
            nc.sync.dma_start(out=sown_t[:, :128], in_=sown.ap())
            selm_t = cpool.tile([128, 512], F32, tag="selm")
            nc.sync.dma_start(out=selm_t[:, :16], in_=selm.ap())
            perm_t = cpool.tile([128, 2048], F32, tag="perm")
            nc.sync.dma_start(out=perm_t[:], in_=perm.ap())

            for _rep in range(repeat):
                canvas = wpool.tile([128, 4096], F32, tag="canvas")
                NCH = 8            # psum column chunks of the canvas
                CCH = CROW // NCH  # cols per chunk
                pstiles = []
                for m in range(NCH):
                    cps = ppool.tile([128, CCH], F32, tag=f"cps{m}", name=f"cps{m}")
                    pstiles.append(cps)
                for j in range(GC):
                    gout = gpool.tile([128, 6656], F32, tag="gout")
                    nc.gpsimd.ap_gather(
                        out_ap=gout[:, :2 * CROW],
                        in_ap=table[:],
                        idxs_ap=gidx_t[:, j * IDXC:(j + 1) * IDXC],
                        channels=128,
                        num_elems=TBL,
                        d=1,
                        num_idxs=2 * CROW,
                    )
                    # Pool-engine fence copy: same-engine ordering guarantees the
                    # gather's SBUF writes are drained before this copy reads them;
                    # PE consumes the copy's output, not the gather's.
                    gout2 = gpool.tile([128, 6656], F32, tag="gout2")
                    nc.gpsimd.tensor_copy(out=gout2[:, :2 * CROW],
                                          in_=gout[:, :2 * CROW])
                    for half in (0, 1):
                        r = 2 * j + half
                        for m in range(NCH):
                            nc.tensor.matmul(
                                out=pstiles[m][:],
                                lhsT=perm_t[:, r * 128:(r + 1) * 128],
                                rhs=gout2[:, half * CROW + m * CCH:
                                          half * CROW + (m + 1) * CCH],
                                start=(r == 0), stop=(r == RR - 1),
                            )
                for m in range(NCH):
                    nc.vector.tensor_copy(out=canvas[:, m * CCH:(m + 1) * CCH],
                                          in_=pstiles[m][:])

                pfx = wpool.tile([128, 4096], F32, tag="pfx")
                if "scan" in ablate:
                    nc.vector.memset(pfx[:, :2], 0.0)
                else:
                    nc.vector._custom_dve(cum_op, out=pfx[:, :CROW], in0=canvas[:, :CROW])
                if debug_taps:
                    nc.sync.dma_start(out=taps["tap_canvas"].ap(), in_=canvas[:, :CROW])
                    nc.sync.dma_start(out=taps["tap_pfx"].ap(), in_=pfx[:, :CROW])

                packed = wpool.tile([128, 1024], F32, tag="packed")
                if "ls" in ablate:
                    nc.vector.memset(packed[:, :2], 0.0)
                elif True:
                    nc.gpsimd.local_scatter(
                    out_ap=packed[:, :NSLOT].bitcast(I16),
                    data_ap=pfx[:, :CROW].bitcast(I16),
                    idxs_ap=bidx_t[:, :2 * CROW],
                    channels=128,
                    num_elems=2 * NSLOT,
                    num_idxs=2 * CROW,
                )

                if debug_taps:
                    nc.sync.dma_start(out=taps["tap_packed"].ap(), in_=packed[:, :NSLOT])
                dif = wpool.tile([128, 1024], F32, tag="dif")
                nc.vector.tensor_copy(out=dif[:, 0:1], in_=packed[:, 0:1])
                nc.vector.tensor_tensor(
                    out=dif[:, 1:NSLOT], in0=packed[:, 1:NSLOT],
                    in1=packed[:, 0:NSLOT - 1], op=mybir.AluOpType.subtract,
                )

                if debug_taps:
                    nc.sync.dma_start(out=taps["tap_dif"].ap(), in_=dif[:, :NSLOT])
                accs = wpool.tile([MCH, 512], F32, tag="accs")
                for m in range(NSLOT // MCH):
                    ps = ppool.tile([MCH, 16], F32, tag=f"cps{m}", name=f"ps{m}")
                    nc.tensor.matmul(
                        out=ps[:], lhsT=dif[:, m * MCH:(m + 1) * MCH],
                        rhs=selm_t[:, :16], start=True, stop=True,
                    )
                    nc.vector.tensor_add(
                        out=accs[:, m * 16:(m + 1) * 16], in0=ps[:],
                        in1=sown_t[:, m * 16:(m + 1) * 16],
                    )

                yt = wpool.tile([MCH, 512], F32, tag="yt")
                nc.vector.tensor_scalar(
                    out=yt[:, :128], in0=accs[:, :128], scalar1=0.0, scalar2=None,
                    op0=mybir.AluOpType.is_gt,
                )
                nc.sync.dma_start(out=y.ap(), in_=yt[:, :128])

    nc.compile()
    return nc


# ---------------------------------------------------------------- host prep
def _permmat():
    pm = np.zeros((128, 16, 128), np.float32)
    for r in range(RR):
        for k in range(K):
            pm[16 * k + r, r, k + 8 * r] = 1.0
    return pm.reshape(128, 16 * 128)


def _selmat():
    m = np.zeros((128, 16), np.float32)
    m[np.arange(128), np.arange(128) // 8] = 1.0
    return m


def prep_core(src_c, dst_c, s, core):
    """src_c: global src ids, dst_c: local dst ids [0,NN); s: full [100000] f32."""
    k = src_c // NN
    srcl = (src_c - k * NN).astype(np.int64)

    cnt = np.bincount(dst_c * K + k, minlength=NN * K).reshape(NN, K)
    absent = cnt == 0
    load_d = cnt.sum(1) + absent.sum(1)
    cum = np.cumsum(load_d)
    total = int(cum[-1])
    targets = total * np.arange(1, RR) / RR
    Rb = np.concatenate([[0], np.searchsorted(cum, targets, side="left") + 1,
                         [NN]]).astype(np.int64)
    rowcounts = np.diff(Rb)
    assert rowcounts.min() > 0 and rowcounts.max() <= NSLOT, rowcounts
    row_of_node = np.repeat(np.arange(RR), rowcounts)

    ad, ak = np.nonzero(absent)
    src_all = np.concatenate([srcl, np.full(len(ad), ZSLOT, np.int64)])
    dst_all = np.concatenate([dst_c, ad])
    k_all = np.concatenate([k, ak])
    row_all = row_of_node[dst_all]
    kr = k_all * RR + row_all
    key = kr * NN + dst_all
    order = np.argsort(key, kind="stable")
    src_s = src_all[order]
    dst_s = dst_all[order]
    key_s = key[order]
    kr_s = kr[order]

    kr_counts = np.bincount(kr_s, minlength=K * RR)
    assert kr_counts.max() <= CROW, kr_counts.max()
    kr_starts = np.concatenate([[0], np.cumsum(kr_counts)])[:-1]
    pos = np.arange(len(kr_s)) - kr_starts[kr_s]

    kk = kr_s // RR
    rr_ = kr_s % RR
    gidx = np.full((128, GC * IDXC), ZSLOT, np.int16)
    ii = (rr_ % 2) * CROW + pos
    gidx[16 * kk + (ii % 16), IDXC * (rr_ // 2) + ii // 16] = src_s.astype(np.int16)

    is_end = np.ones(len(key_s), bool)
    is_end[:-1] = key_s[1:] != key_s[:-1]
    e = np.nonzero(is_end)[0]
    e_k, e_r = kr_s[e] // RR, kr_s[e] % RR
    slot = (dst_s[e] - Rb[e_r]).astype(np.int64)
    bidx = np.full((128, 2 * CROW), -1, np.int16)
    bp = e_k + 8 * e_r
    bidx[bp, 2 * pos[e]] = (2 * slot).astype(np.int16)
    bidx[bp, 2 * pos[e] + 1] = (2 * slot + 1).astype(np.int16)

    s_own = s[core * NN:(core + 1) * NN]
    sown = np.zeros((MCH, 128), np.float32)
    for r in range(RR):
        n0, n1 = int(Rb[r]), int(Rb[r + 1])
        for m in range(NSLOT // MCH):
            lo = n0 + m * MCH
            c_ = min(MCH, n1 - lo)
            if c_ > 0:
                sown[:c_, m * 16 + r] = s_own[lo:lo + c_]
    return gidx, bidx, sown, Rb


def decode_core(yc, Rb):
    out = np.zeros(NN, np.int64)
    for r in range(RR):
        n0, n1 = int(Rb[r]), int(Rb[r + 1])
        for m in range(NSLOT // MCH):
            lo = n0 + m * MCH
            c_ = min(MCH, n1 - lo)
            if c_ > 0:
                out[lo:lo + c_] = (yc[:c_, m * 16 + r] > 0.5).astype(np.int64)
    return out


def prep_all(x, edge_index):
    s = np.asarray(x[:, 0], np.float32)
    src = np.asarray(edge_index[0], np.int64)
    dst = np.asarray(edge_index[1], np.int64)
    owner = dst // NN
    sel_order = np.argsort(owner, kind="stable")
    bounds = np.searchsorted(owner[sel_order], np.arange(N_CORES + 1))
    selm = _selmat()
    permm = _permmat()
    s_chunks = np.zeros((K, TBL), np.float32)
    s_chunks[:, :NN] = s.reshape(K, NN)
    in_maps, infos = [], []
    for c in range(N_CORES):
        idx = sel_order[bounds[c]:bounds[c + 1]]
        gidx, bidx, sown, Rb = prep_core(src[idx], dst[idx] - c * NN, s, c)
        in_maps.append({
            "s_chunks": s_chunks, "gidx": gidx, "bidx": bidx,
            "sown": sown, "selm": selm, "perm": permm,
        })
        infos.append(Rb)
    return in_maps, infos


def decode_all(results, infos):
    return np.concatenate(
        [decode_core(results[c]["y"], infos[c]) for c in range(N_CORES)])


# ------------------------------------------------------------- numpy model
def numpy_model_core(in_map):
    """Bit-for-bit-ish model of the device pipeline for one core (f32 order
    matches: sequential scan, diffs, 8-way group sum)."""
    s_chunks = in_map["s_chunks"]
    table = np.zeros((128, TBL), np.float32)
    for p in range(128):
        table[p, :] = s_chunks[p // 16]
    gidx = in_map["gidx"]
    canvas = np.zeros((128, CROW), np.float32)
    for r in range(RR):
        sl = gidx[:, (r // 2) * IDXC:(r // 2 + 1) * IDXC]
        for k in range(K):
            idxs = sl[16 * k:16 * (k + 1), :].T.reshape(-1)  # (s p), 2*CROW long
            idxs = idxs[(r % 2) * CROW:(r % 2 + 1) * CROW]
            canvas[k + 8 * r, :] = table[16 * k + r, idxs]
    pfx = np.cumsum(canvas, axis=1, dtype=np.float32)
    packed = np.zeros((128, NSLOT), np.float32)
    pk16 = packed.view(np.int16).reshape(128, 2 * NSLOT)
    pf16 = pfx.view(np.int16).reshape(128, 2 * CROW)
    bidx = in_map["bidx"]
    for p in range(128):
        v = bidx[p] >= 0
        pk16[p, bidx[p][v].astype(np.int64)] = pf16[p, np.nonzero(v)[0]]
    dif = np.zeros((128, NSLOT), np.float32)
    dif[:, 0] = packed[:, 0]
    dif[:, 1:] = packed[:, 1:] - packed[:, :-1]
    accs = np.zeros((MCH, 128), np.float32)
    selm = in_map["selm"]
    for m in range(NSLOT // MCH):
        ps = dif[:, m * MCH:(m + 1) * MCH].T @ selm
        accs[:, m * 16:(m + 1) * 16] = ps + in_map["sown"][:, m * 16:(m + 1) * 16]
    return (accs > 0).astype(np.float32)


# ---------------------------------------------------------------- entrypoint
_NC_CACHE = {}


def kernel(x, edge_index):
    """Full inputs in, full output out; shards across 8 NeuronCores inside."""
    from concourse.bass_utils import run_bass_kernel_spmd
    x = np.asarray(x)
    edge_index = np.asarray(edge_index)
    in_maps, infos = prep_all(x, edge_index)
    if "nc" not in _NC_CACHE:
        _NC_CACHE["nc"] = build_nc(num_devices=N_CORES)
    res = run_bass_kernel_spmd(_NC_CACHE["nc"], in_maps,
                               core_ids=list(range(N_CORES)))
    out = decode_all(res.results, infos)
    return out.astype(np.int64)



# revision 8
# speedup vs baseline: 243.7789x; 243.7789x over previous
"""GNN scatter-mean via local_scatter expansion (no ap_gather).

Per NeuronCore c (8 total): owns dsts [12500c, 12500(c+1)); edges bucketed by
dst owner. Only column 0 of x matters (deg>=1 => mean sign == sum sign).

Partition p = 16k + r: src chunk k = src//12500 (8 chunks), dst range r of 16
(fixed rowcounts 782*4 + 781*12, RB boundaries). Srcs within a chunk split
into B=6 bands. Per (p, band): stream = real edges (src in chunk k, band b,
dst in range r) ordered by dst slot; every (slot, band) without an edge
(incl. pad slots) contributes one *unwritten* (zero) dummy position forming
its own run, so the run grid is fully populated.

Device, per rep:
  C1_b  local_scatter: per-partition packed table (each needed (p,src) value
        once, mult-sorted desc; device-built prefix-dup regions serve
        occurrences t>=2) -> canvas band window, at dst-sorted positions.
  cumsum_b (custom DVE scan) per band window (band-local prefix sums).
  C3_b  local_scatter: pfx at run-end positions -> grid[:, b*G:(b+1)*G].
  diff along grid; band-boundary columns b*G overwritten with grid copy
  (band-local scans); sum 6 bands; DMA-fold 8 chunks into 16 partitions;
  7 adds + s_own; threshold; DMA out y [16, G].
"""
import numpy as np

import concourse.bass as bass
import concourse.bacc as bacc
import concourse.mybir as mybir
import concourse.tile as tile

N_NODES = 100000
N_CORES = 8
NN = N_NODES // N_CORES      # 12500
K = 8                        # src chunks
RR = 16                      # dst ranges (partitions per chunk)
B = 6                        # src bands per chunk
GSL = 782                    # grid band stride (max rowcount)
G6 = B * GSL
ROWCOUNTS = np.asarray([782] * 4 + [781] * 12)
RB = np.concatenate([[0], np.cumsum(ROWCOUNTS)]).astype(np.int64)
BANDQ = np.linspace(0, NN, B + 1).astype(np.int64)

F32 = mybir.dt.float32
I16 = mybir.dt.int16


# --------------------------------------------------------------- geometry
def _band_mult(src_c, dst_c):
    """Per-partition per-src multiplicity [128, NN] for one core."""
    k = (src_c // NN).astype(np.int64)
    r = np.searchsorted(RB, dst_c, side="right") - 1
    p = 16 * k + r
    inchunk = src_c - k * NN
    mult = np.zeros((128, NN), np.int32)
    np.add.at(mult, (p, inchunk), 1)
    return mult, p, inchunk, r


# ---------------------------------------------------------------- host prep
def prep_all(x, edge_index):
    s = np.asarray(x[:, 0], np.float32)
    src = np.asarray(edge_index[0], np.int64)
    dst = np.asarray(edge_index[1], np.int64)
    owner = dst // NN
    sel_order = np.argsort(owner, kind="stable")
    bounds = np.searchsorted(owner[sel_order], np.arange(N_CORES + 1))

    cores = []
    for c in range(N_CORES):
        idx = sel_order[bounds[c]:bounds[c + 1]]
        sc = src[idx]
        dc = dst[idx] - c * NN
        mult, p, inchunk, r = _band_mult(sc, dc)
        slot = dc - RB[r]
        band = np.searchsorted(BANDQ, inchunk, side="right") - 1
        cores.append(dict(sc=sc, dc=dc, mult=mult, p=p, inchunk=inchunk,
                          slot=slot, band=band))

    # ---- global geometry
    NT = []   # per band: [n_1, n_2, ...]
    W = []
    for b in range(B):
        lo, hi = int(BANDQ[b]), int(BANDQ[b + 1])
        nts = []
        t = 1
        while True:
            nt = max(int((cd["mult"][:, lo:hi] >= t).sum(1).max())
                     for cd in cores)
            if nt == 0:
                break
            nts.append(nt)
            t += 1
        NT.append(nts)
        wb = 0
        for cd in cores:
            m = cd["band"] == b
            pb, sb = cd["p"][m], cd["slot"][m]
            cnt_ps = np.zeros((128, GSL), np.int32)
            np.add.at(cnt_ps, (pb, sb), 1)
            cd[f"cnt_ps_{b}"] = cnt_ps
            entries = np.maximum(cnt_ps, 1)
            wb = max(wb, int(entries.sum(1).max()))
        wb += wb % 2
        assert wb <= 1023, (b, wb)
        W.append(wb)

    NB = []   # C1 band data width (f32): n_1 + n_2 + ... (dup regions)
    DUP = []  # per band: list of (dest_off, length) prefix copies
    for b in range(B):
        nts = NT[b]
        npos = nts[0]
        dups = []
        for nt in nts[1:]:
            dups.append((npos, nt))
            npos += nt
        NB.append(npos)
        DUP.append(dups)

    S = np.concatenate([[0], np.cumsum(W)]).astype(int)
    SL = int(S[-1])
    TS = np.concatenate([[0], np.cumsum(NB)]).astype(int)
    TBW = int(TS[-1])
    O1 = np.concatenate([[0], np.cumsum([2 * n for n in NB])]).astype(int)

    geom = dict(W=[int(w) for w in W], NB=[int(n) for n in NB],
                DUP=DUP, TBW=TBW, SL=SL,
                S=[int(v) for v in S], TS=[int(v) for v in TS],
                O1=[int(v) for v in O1])

    # ---- per-core arrays
    in_maps = []
    for c, cd in enumerate(cores):
        tin = np.zeros((128, TBW), np.float32)
        c1x = np.full((128, int(O1[-1])), -1, np.int16)
        c3x = np.full((128, 2 * SL), -1, np.int16)

        # occurrence rank per (p, src) in stream order (band, slot, arrival)
        E = len(cd["p"])
        eord = np.lexsort((np.arange(E), cd["slot"], cd["band"], cd["p"]))
        pe = cd["p"][eord]; be = cd["band"][eord]; se = cd["slot"][eord]
        ice = cd["inchunk"][eord]
        pi = pe * NN + ice
        so = np.argsort(pi, kind="stable")
        cnts = np.bincount(pi, minlength=128 * NN)
        cnts = cnts[cnts > 0]
        occ_sorted = np.arange(E) - np.repeat(
            np.concatenate([[0], np.cumsum(cnts)])[:-1], cnts)
        occ = np.empty(E, np.int64)
        occ[so] = occ_sorted

        for b in range(B):
            lo, hi = int(BANDQ[b]), int(BANDQ[b + 1])
            bw = hi - lo
            mb = cd["mult"][:, lo:hi]
            order = np.argsort(-mb, axis=1, kind="stable")
            inv_order = np.empty_like(order)
            np.put_along_axis(
                inv_order, order,
                np.tile(np.arange(bw), (128, 1)), axis=1)
            chunk_base = (np.arange(128) // 16) * NN
            vals = s[(chunk_base[:, None] + lo + order)].astype(np.float32)
            ncopy = min(NB[b], bw)
            tin[:, TS[b]:TS[b] + ncopy] = vals[:, :ncopy]

            cnt_ps = cd[f"cnt_ps_{b}"]
            entries = np.maximum(cnt_ps, 1)
            run_start = np.concatenate(
                [np.zeros((128, 1), np.int64),
                 np.cumsum(entries, 1)[:, :-1].astype(np.int64)], 1)
            run_end = run_start + entries - 1

            m = be == b
            pb, sb, icb, occb = pe[m], se[m], ice[m], occ[m]
            # rank within (p, slot) run
            key_ps = pb * GSL + sb
            so2 = np.argsort(key_ps, kind="stable")
            cnts2 = np.bincount(key_ps, minlength=128 * GSL)
            cnts2 = cnts2[cnts2 > 0]
            rank_sorted = np.arange(len(pb)) - np.repeat(
                np.concatenate([[0], np.cumsum(cnts2)])[:-1], cnts2)
            rank = np.empty(len(pb), np.int64)
            rank[so2] = rank_sorted
            pos = run_start[pb, sb] + rank          # stream pos within band

            jpos = inv_order[pb, icb - lo]
            starts = np.zeros(len(NT[b]) + 1, np.int64)
            for t in range(1, len(NT[b])):
                starts[t] = DUP[b][t - 1][0]
            tabpos = starts[occb] + jpos
            assert (tabpos < NB[b]).all()
            c1x[pb, O1[b] + 2 * tabpos] = (2 * pos).astype(np.int16)
            c1x[pb, O1[b] + 2 * tabpos + 1] = (2 * pos + 1).astype(np.int16)

            pp = np.repeat(np.arange(128), GSL)
            ss = np.tile(np.arange(GSL), 128)
            repos = run_end.reshape(-1)
            c3x[pp, 2 * (S[b] + repos)] = (2 * ss).astype(np.int16)
            c3x[pp, 2 * (S[b] + repos) + 1] = (2 * ss + 1).astype(np.int16)

        sown = np.zeros((RR, GSL), np.float32)
        s_own = s[c * NN:(c + 1) * NN]
        for rr in range(RR):
            n0, n1_ = int(RB[rr]), int(RB[rr + 1])
            sown[rr, :n1_ - n0] = s_own[n0:n1_]

        in_maps.append({"tin": tin, "c1x": c1x, "c3x": c3x, "sown": sown})
    return geom, in_maps


# ------------------------------------------------------------- numpy model
def numpy_model_core(geom, im):
    W, NB, DUP = geom["W"], geom["NB"], geom["DUP"]
    S, TS, O1 = geom["S"], geom["TS"], geom["O1"]
    SL = geom["SL"]
    # device dup-region build
    wtab = im["tin"].copy()
    for b in range(B):
        for (doff, dlen) in DUP[b]:
            wtab[:, TS[b] + doff:TS[b] + doff + dlen] = \
                wtab[:, TS[b]:TS[b] + dlen]
    canvas = np.zeros((128, SL), np.float32)
    cv16 = canvas.view(np.int16)
    wt16 = wtab.view(np.int16)
    c1x = im["c1x"]
    for b in range(B):
        # local_scatter zeroes its window then writes
        cv16[:, 2 * S[b]:2 * (S[b] + W[b])] = 0
        for pp in range(128):
            idx = c1x[pp, O1[b]:O1[b + 1]]
            v = idx >= 0
            cv16[pp, 2 * S[b] + idx[v].astype(np.int64)] = \
                wt16[pp, 2 * TS[b] + np.nonzero(v)[0]]
    pfx = np.zeros((128, SL), np.float32)
    for b in range(B):
        pfx[:, S[b]:S[b + 1]] = np.cumsum(
            canvas[:, S[b]:S[b + 1]], axis=1, dtype=np.float32)
    grid = np.zeros((128, G6), np.float32)
    g16 = grid.view(np.int16)
    p16 = pfx.view(np.int16)
    c3x = im["c3x"]
    for b in range(B):
        g16[:, 2 * b * GSL:2 * (b + 1) * GSL] = 0
        for pp in range(128):
            idx = c3x[pp, 2 * S[b]:2 * S[b + 1]]
            v = idx >= 0
            g16[pp, 2 * b * GSL + idx[v].astype(np.int64)] = \
                p16[pp, 2 * S[b] + np.nonzero(v)[0]]
    dif = np.zeros((128, G6), np.float32)
    dif[:, 0] = grid[:, 0]
    dif[:, 1:] = grid[:, 1:] - grid[:, :-1]
    for b in range(B):
        dif[:, b * GSL] = grid[:, b * GSL]
    acc = dif[:, 0:GSL].copy()
    for b in range(1, B):
        acc = acc + dif[:, b * GSL:(b + 1) * GSL]
    racc = np.zeros((RR, GSL), np.float32)
    for k in range(K):
        racc = racc + acc[16 * k:16 * (k + 1), :]
    racc = racc + im["sown"]
    return (racc > 0).astype(np.float32)


def decode_core(yc):
    out = np.zeros(NN, np.int64)
    for rr in range(RR):
        n0, n1_ = int(RB[rr]), int(RB[rr + 1])
        out[n0:n1_] = (yc[rr, :n1_ - n0] > 0.5).astype(np.int64)
    return out


# ---------------------------------------------------------------- device IR
def build_nc(geom, num_devices=N_CORES, repeat=1, hwloop=False, ablate=()):
    W, NB, DUP = geom["W"], geom["NB"], geom["DUP"]
    S, TS, O1 = geom["S"], geom["TS"], geom["O1"]
    SL, TBW = geom["SL"], geom["TBW"]

    nc = bacc.Bacc("TRN2", target_bir_lowering=False, debug=False,
                   num_devices=num_devices)
    tin = nc.dram_tensor("tin", [128, TBW], F32, kind="ExternalInput")
    c1x = nc.dram_tensor("c1x", [128, O1[-1]], I16, kind="ExternalInput")
    c3x = nc.dram_tensor("c3x", [128, 2 * SL], I16, kind="ExternalInput")
    sown = nc.dram_tensor("sown", [RR, GSL], F32, kind="ExternalInput")
    y = nc.dram_tensor("y", [RR, GSL], F32, kind="ExternalOutput")

    with tile.TileContext(nc) as tc:
        with (
            tc.tile_pool(name="const", bufs=1) as cpool,
            tc.tile_pool(name="work", bufs=1) as wpool,
        ):
            wtab = cpool.tile([128, TBW], F32, tag="wtab")
            nc.sync.dma_start(out=wtab[:], in_=tin.ap())
            c1x_t = cpool.tile([128, O1[-1]], I16, tag="c1x")
            nc.sync.dma_start(out=c1x_t[:], in_=c1x.ap())
            c3x_t = cpool.tile([128, 2 * SL], I16, tag="c3x")
            nc.sync.dma_start(out=c3x_t[:], in_=c3x.ap())
            sown_t = cpool.tile([RR, GSL], F32, tag="sown")
            nc.sync.dma_start(out=sown_t[:], in_=sown.ap())

            for b in range(B):
                for (doff, dlen) in DUP[b]:
                    nc.vector.tensor_copy(
                        out=wtab[:, TS[b] + doff:TS[b] + doff + dlen],
                        in_=wtab[:, TS[b]:TS[b] + dlen])

            from contextlib import ExitStack
            with ExitStack() as stk:
                if hwloop and repeat > 1:
                    stk.enter_context(tc.For_i(0, repeat))
                    reps = (0,)
                else:
                    reps = range(repeat)
                for _rep in reps:
                    canvas = wpool.tile([128, SL], F32, tag="canvas")
                    pfx = wpool.tile([128, SL], F32, tag="pfx")
                    grid = wpool.tile([128, G6], F32, tag="grid")
                    for b in range(B):
                        if "c1" in ablate:
                            nc.vector.memset(canvas[:, S[b]:S[b] + 2], 0.0)
                        else:
                            nc.gpsimd.local_scatter(
                                out_ap=canvas[:, S[b]:S[b + 1]].bitcast(I16),
                                data_ap=wtab[:, TS[b]:TS[b + 1]].bitcast(I16),
                                idxs_ap=c1x_t[:, O1[b]:O1[b + 1]],
                                channels=128,
                                num_elems=2 * W[b],
                                num_idxs=2 * NB[b],
                            )
                        if "scan" in ablate:
                            nc.vector.memset(pfx[:, S[b]:S[b] + 2], 0.0)
                        else:
                            nc.vector.tensor_tensor_scan(
                                out=pfx[:, S[b]:S[b + 1]],
                                data0=canvas[:, S[b]:S[b + 1]],
                                data1=canvas[:, S[b]:S[b + 1]],
                                initial=0.0,
                                op0=mybir.AluOpType.add,
                                op1=mybir.AluOpType.bypass)
                    for b in range(B):
                        if "c3" in ablate:
                            nc.vector.memset(grid[:, b * GSL:b * GSL + 2], 0.0)
                        else:
                            nc.gpsimd.local_scatter(
                                out_ap=grid[:, b * GSL:(b + 1) * GSL].bitcast(I16),
                                data_ap=pfx[:, S[b]:S[b + 1]].bitcast(I16),
                                idxs_ap=c3x_t[:, 2 * S[b]:2 * S[b + 1]],
                                channels=128,
                                num_elems=2 * GSL,
                                num_idxs=2 * W[b],
                            )
                    dif = wpool.tile([128, G6], F32, tag="dif")
                    nc.vector.tensor_copy(out=dif[:, 0:1], in_=grid[:, 0:1])
                    nc.vector.tensor_tensor(
                        out=dif[:, 1:G6], in0=grid[:, 1:G6],
                        in1=grid[:, 0:G6 - 1], op=mybir.AluOpType.subtract)
                    for b in range(1, B):
                        nc.vector.tensor_copy(
                            out=dif[:, b * GSL:b * GSL + 1],
                            in_=grid[:, b * GSL:b * GSL + 1])
                    acc = wpool.tile([128, GSL], F32, tag="acc")
                    nc.vector.tensor_add(out=acc[:], in0=dif[:, 0:GSL],
                                         in1=dif[:, GSL:2 * GSL])
                    for b in range(2, B):
                        nc.vector.tensor_add(
                            out=acc[:], in0=acc[:],
                            in1=dif[:, b * GSL:(b + 1) * GSL])
                    redu = wpool.tile([RR, K * GSL], F32, tag="redu")
                    for k in range(K):
                        nc.sync.dma_start(
                            out=redu[0:RR, k * GSL:(k + 1) * GSL],
                            in_=acc[16 * k:16 * (k + 1), :])
                    racc = wpool.tile([RR, GSL], F32, tag="racc")
                    nc.vector.tensor_add(out=racc[:], in0=redu[:, 0:GSL],
                                         in1=redu[:, GSL:2 * GSL])
                    for k in range(2, K):
                        nc.vector.tensor_add(
                            out=racc[:], in0=racc[:],
                            in1=redu[:, k * GSL:(k + 1) * GSL])
                    nc.vector.tensor_add(out=racc[:], in0=racc[:],
                                         in1=sown_t[:])
                    yt = wpool.tile([RR, GSL], F32, tag="yt")
                    nc.vector.tensor_scalar(
                        out=yt[:], in0=racc[:], scalar1=0.0, scalar2=None,
                        op0=mybir.AluOpType.is_gt)
                    nc.sync.dma_start(out=y.ap(), in_=yt[:])
    nc.compile()
    return nc


# ---------------------------------------------------------------- entrypoint
_NC_CACHE = {}


def kernel(x, edge_index):
    from concourse.bass_utils import run_bass_kernel_spmd
    x = np.asarray(x)
    edge_index = np.asarray(edge_index)
    geom, in_maps = prep_all(x, edge_index)
    key = (tuple(geom["W"]), tuple(geom["NB"]))
    if key not in _NC_CACHE:
        _NC_CACHE[key] = build_nc(geom, num_devices=N_CORES)
    res = run_bass_kernel_spmd(_NC_CACHE[key], in_maps,
                               core_ids=list(range(N_CORES)))
    out = np.concatenate(
        [decode_core(res.results[c]["y"]) for c in range(N_CORES)])
    return out.astype(np.int64)


# revision 9
# speedup vs baseline: 347.4367x; 1.4252x over previous
"""GNN scatter-mean via local_scatter expansion (no ap_gather).

Per NeuronCore c (8 total): owns dsts [12500c, 12500(c+1)); edges bucketed by
dst owner. Only column 0 of x matters (deg>=1 => mean sign == sum sign).

Partition p = 16k + r: src chunk k = src//12500 (8 chunks), dst range r of 16
(fixed rowcounts 782*4 + 781*12, RB boundaries). Srcs within a chunk split
into B=6 bands. Per (p, band): stream = real edges (src in chunk k, band b,
dst in range r) ordered by dst slot; every (slot, band) without an edge
(incl. pad slots) contributes one *unwritten* (zero) dummy position forming
its own run, so the run grid is fully populated.

Device, per rep:
  C1_b  local_scatter: per-partition packed table (each needed (p,src) value
        once, mult-sorted desc; device-built prefix-dup regions serve
        occurrences t>=2) -> canvas band window, at dst-sorted positions.
  cumsum_b (native DVE tensor_tensor_scan, add/bypass) per band window
        (band-local prefix sums).
  C3_b  local_scatter: pfx at run-end positions -> grid[:, b*G:(b+1)*G].
  diff along grid; band-boundary columns b*G overwritten with grid copy
  (band-local scans); sum 6 bands; DMA-fold 8 chunks into 16 partitions;
  7 adds + s_own; threshold; DMA out y [16, G].
"""
import numpy as np

import concourse.bacc as bacc
import concourse.mybir as mybir
import concourse.tile as tile

N_NODES = 100000
N_CORES = 8
NN = N_NODES // N_CORES      # 12500
K = 8                        # src chunks
RR = 16                      # dst ranges (partitions per chunk)
B = 6                        # src bands per chunk
GSL = 782                    # grid band stride (max rowcount)
G6 = B * GSL
ROWCOUNTS = np.asarray([782] * 4 + [781] * 12)
RB = np.concatenate([[0], np.cumsum(ROWCOUNTS)]).astype(np.int64)
BANDQ = np.linspace(0, NN, B + 1).astype(np.int64)

F32 = mybir.dt.float32
I16 = mybir.dt.int16


# --------------------------------------------------------------- geometry
def _band_mult(src_c, dst_c):
    """Per-partition per-src multiplicity [128, NN] for one core."""
    k = (src_c // NN).astype(np.int64)
    r = np.searchsorted(RB, dst_c, side="right") - 1
    p = 16 * k + r
    inchunk = src_c - k * NN
    mult = np.zeros((128, NN), np.int32)
    np.add.at(mult, (p, inchunk), 1)
    return mult, p, inchunk, r


# ---------------------------------------------------------------- host prep
def prep_all(x, edge_index):
    s = np.asarray(x[:, 0], np.float32)
    src = np.asarray(edge_index[0], np.int64)
    dst = np.asarray(edge_index[1], np.int64)
    owner = dst // NN
    sel_order = np.argsort(owner, kind="stable")
    bounds = np.searchsorted(owner[sel_order], np.arange(N_CORES + 1))

    cores = []
    for c in range(N_CORES):
        idx = sel_order[bounds[c]:bounds[c + 1]]
        sc = src[idx]
        dc = dst[idx] - c * NN
        mult, p, inchunk, r = _band_mult(sc, dc)
        slot = dc - RB[r]
        band = np.searchsorted(BANDQ, inchunk, side="right") - 1
        cores.append(dict(sc=sc, dc=dc, mult=mult, p=p, inchunk=inchunk,
                          slot=slot, band=band))

    # ---- global geometry
    NT = []   # per band: [n_1, n_2, ...]
    W = []
    for b in range(B):
        lo, hi = int(BANDQ[b]), int(BANDQ[b + 1])
        nts = []
        t = 1
        while True:
            nt = max(int((cd["mult"][:, lo:hi] >= t).sum(1).max())
                     for cd in cores)
            if nt == 0:
                break
            nts.append(nt)
            t += 1
        NT.append(nts)
        wb = 0
        for cd in cores:
            m = cd["band"] == b
            pb, sb = cd["p"][m], cd["slot"][m]
            cnt_ps = np.zeros((128, GSL), np.int32)
            np.add.at(cnt_ps, (pb, sb), 1)
            cd[f"cnt_ps_{b}"] = cnt_ps
            entries = np.maximum(cnt_ps, 1)
            wb = max(wb, int(entries.sum(1).max()))
        wb += wb % 2
        assert wb <= 1023, (b, wb)
        W.append(wb)

    NB = []   # C1 band data width (f32): n_1 + n_2 + ... (dup regions)
    DUP = []  # per band: list of (dest_off, length) prefix copies
    for b in range(B):
        nts = NT[b]
        npos = nts[0]
        dups = []
        for nt in nts[1:]:
            dups.append((npos, nt))
            npos += nt
        NB.append(npos)
        DUP.append(dups)

    S = np.concatenate([[0], np.cumsum(W)]).astype(int)
    SL = int(S[-1])
    TS = np.concatenate([[0], np.cumsum(NB)]).astype(int)
    TBW = int(TS[-1])
    O1 = np.concatenate([[0], np.cumsum([2 * n for n in NB])]).astype(int)

    geom = dict(W=[int(w) for w in W], NB=[int(n) for n in NB],
                DUP=DUP, TBW=TBW, SL=SL,
                S=[int(v) for v in S], TS=[int(v) for v in TS],
                O1=[int(v) for v in O1])

    # ---- per-core arrays
    in_maps = []
    for c, cd in enumerate(cores):
        tin = np.zeros((128, TBW), np.float32)
        c1x = np.full((128, int(O1[-1])), -1, np.int16)
        c3x = np.full((128, 2 * SL), -1, np.int16)

        # occurrence rank per (p, src) in stream order (band, slot, arrival)
        E = len(cd["p"])
        eord = np.lexsort((np.arange(E), cd["slot"], cd["band"], cd["p"]))
        pe = cd["p"][eord]; be = cd["band"][eord]; se = cd["slot"][eord]
        ice = cd["inchunk"][eord]
        pi = pe * NN + ice
        so = np.argsort(pi, kind="stable")
        cnts = np.bincount(pi, minlength=128 * NN)
        cnts = cnts[cnts > 0]
        occ_sorted = np.arange(E) - np.repeat(
            np.concatenate([[0], np.cumsum(cnts)])[:-1], cnts)
        occ = np.empty(E, np.int64)
        occ[so] = occ_sorted

        for b in range(B):
            lo, hi = int(BANDQ[b]), int(BANDQ[b + 1])
            bw = hi - lo
            mb = cd["mult"][:, lo:hi]
            order = np.argsort(-mb, axis=1, kind="stable")
            inv_order = np.empty_like(order)
            np.put_along_axis(
                inv_order, order,
                np.tile(np.arange(bw), (128, 1)), axis=1)
            chunk_base = (np.arange(128) // 16) * NN
            vals = s[(chunk_base[:, None] + lo + order)].astype(np.float32)
            ncopy = min(NB[b], bw)
            tin[:, TS[b]:TS[b] + ncopy] = vals[:, :ncopy]

            cnt_ps = cd[f"cnt_ps_{b}"]
            entries = np.maximum(cnt_ps, 1)
            run_start = np.concatenate(
                [np.zeros((128, 1), np.int64),
                 np.cumsum(entries, 1)[:, :-1].astype(np.int64)], 1)
            run_end = run_start + entries - 1

            m = be == b
            pb, sb, icb, occb = pe[m], se[m], ice[m], occ[m]
            # rank within (p, slot) run
            key_ps = pb * GSL + sb
            so2 = np.argsort(key_ps, kind="stable")
            cnts2 = np.bincount(key_ps, minlength=128 * GSL)
            cnts2 = cnts2[cnts2 > 0]
            rank_sorted = np.arange(len(pb)) - np.repeat(
                np.concatenate([[0], np.cumsum(cnts2)])[:-1], cnts2)
            rank = np.empty(len(pb), np.int64)
            rank[so2] = rank_sorted
            pos = run_start[pb, sb] + rank          # stream pos within band

            jpos = inv_order[pb, icb - lo]
            starts = np.zeros(len(NT[b]) + 1, np.int64)
            for t in range(1, len(NT[b])):
                starts[t] = DUP[b][t - 1][0]
            tabpos = starts[occb] + jpos
            assert (tabpos < NB[b]).all()
            c1x[pb, O1[b] + 2 * tabpos] = (2 * pos).astype(np.int16)
            c1x[pb, O1[b] + 2 * tabpos + 1] = (2 * pos + 1).astype(np.int16)

            pp = np.repeat(np.arange(128), GSL)
            ss = np.tile(np.arange(GSL), 128)
            repos = run_end.reshape(-1)
            c3x[pp, 2 * (S[b] + repos)] = (2 * ss).astype(np.int16)
            c3x[pp, 2 * (S[b] + repos) + 1] = (2 * ss + 1).astype(np.int16)

        sown = np.zeros((RR, GSL), np.float32)
        s_own = s[c * NN:(c + 1) * NN]
        for rr in range(RR):
            n0, n1_ = int(RB[rr]), int(RB[rr + 1])
            sown[rr, :n1_ - n0] = s_own[n0:n1_]

        in_maps.append({"tin": tin, "c1x": c1x, "c3x": c3x, "sown": sown})
    return geom, in_maps


# ------------------------------------------------------------- numpy model
def numpy_model_core(geom, im):
    W, NB, DUP = geom["W"], geom["NB"], geom["DUP"]
    S, TS, O1 = geom["S"], geom["TS"], geom["O1"]
    SL = geom["SL"]
    # device dup-region build
    wtab = im["tin"].copy()
    for b in range(B):
        for (doff, dlen) in DUP[b]:
            wtab[:, TS[b] + doff:TS[b] + doff + dlen] = \
                wtab[:, TS[b]:TS[b] + dlen]
    canvas = np.zeros((128, SL), np.float32)
    cv16 = canvas.view(np.int16)
    wt16 = wtab.view(np.int16)
    c1x = im["c1x"]
    for b in range(B):
        # local_scatter zeroes its window then writes
        cv16[:, 2 * S[b]:2 * (S[b] + W[b])] = 0
        for pp in range(128):
            idx = c1x[pp, O1[b]:O1[b + 1]]
            v = idx >= 0
            cv16[pp, 2 * S[b] + idx[v].astype(np.int64)] = \
                wt16[pp, 2 * TS[b] + np.nonzero(v)[0]]
    pfx = np.zeros((128, SL), np.float32)
    for b in range(B):
        pfx[:, S[b]:S[b + 1]] = np.cumsum(
            canvas[:, S[b]:S[b + 1]], axis=1, dtype=np.float32)
    grid = np.zeros((128, G6), np.float32)
    g16 = grid.view(np.int16)
    p16 = pfx.view(np.int16)
    c3x = im["c3x"]
    for b in range(B):
        g16[:, 2 * b * GSL:2 * (b + 1) * GSL] = 0
        for pp in range(128):
            idx = c3x[pp, 2 * S[b]:2 * S[b + 1]]
            v = idx >= 0
            g16[pp, 2 * b * GSL + idx[v].astype(np.int64)] = \
                p16[pp, 2 * S[b] + np.nonzero(v)[0]]
    dif = np.zeros((128, G6), np.float32)
    dif[:, 0] = grid[:, 0]
    dif[:, 1:] = grid[:, 1:] - grid[:, :-1]
    for b in range(B):
        dif[:, b * GSL] = grid[:, b * GSL]
    acc = dif[:, 0:GSL].copy()
    for b in range(1, B):
        acc = acc + dif[:, b * GSL:(b + 1) * GSL]
    racc = np.zeros((RR, GSL), np.float32)
    for k in range(K):
        racc = racc + acc[16 * k:16 * (k + 1), :]
    racc = racc + im["sown"]
    return (racc > 0).astype(np.float32)


def decode_core(yc):
    out = np.zeros(NN, np.int64)
    for rr in range(RR):
        n0, n1_ = int(RB[rr]), int(RB[rr + 1])
        out[n0:n1_] = (yc[rr, :n1_ - n0] > 0.5).astype(np.int64)
    return out


# ---------------------------------------------------------------- device IR
def build_nc(geom, num_devices=N_CORES, repeat=1, hwloop=False, ablate=()):
    W, NB, DUP = geom["W"], geom["NB"], geom["DUP"]
    S, TS, O1 = geom["S"], geom["TS"], geom["O1"]
    SL, TBW = geom["SL"], geom["TBW"]

    nc = bacc.Bacc("TRN2", target_bir_lowering=False, debug=False,
                   num_devices=num_devices)
    tin = nc.dram_tensor("tin", [128, TBW], F32, kind="ExternalInput")
    c1x = nc.dram_tensor("c1x", [128, O1[-1]], I16, kind="ExternalInput")
    c3x = nc.dram_tensor("c3x", [128, 2 * SL], I16, kind="ExternalInput")
    sown = nc.dram_tensor("sown", [RR, GSL], F32, kind="ExternalInput")
    y = nc.dram_tensor("y", [RR, GSL], F32, kind="ExternalOutput")

    with tile.TileContext(nc) as tc:
        with (
            tc.tile_pool(name="const", bufs=1) as cpool,
            tc.tile_pool(name="work", bufs=1) as wpool,
        ):
            wtab = cpool.tile([128, TBW], F32, tag="wtab")
            nc.sync.dma_start(out=wtab[:], in_=tin.ap())
            c1x_t = cpool.tile([128, O1[-1]], I16, tag="c1x")
            nc.sync.dma_start(out=c1x_t[:], in_=c1x.ap())
            c3x_t = cpool.tile([128, 2 * SL], I16, tag="c3x")
            nc.sync.dma_start(out=c3x_t[:], in_=c3x.ap())
            sown_t = cpool.tile([RR, GSL], F32, tag="sown")
            nc.sync.dma_start(out=sown_t[:], in_=sown.ap())

            for b in range(B):
                for (doff, dlen) in DUP[b]:
                    nc.vector.tensor_copy(
                        out=wtab[:, TS[b] + doff:TS[b] + doff + dlen],
                        in_=wtab[:, TS[b]:TS[b] + dlen])

            from contextlib import ExitStack
            with ExitStack() as stk:
                if hwloop and repeat > 1:
                    stk.enter_context(tc.For_i(0, repeat))
                    reps = (0,)
                else:
                    reps = range(repeat)
                for _rep in reps:
                    canvas = wpool.tile([128, SL], F32, tag="canvas")
                    pfx = wpool.tile([128, SL], F32, tag="pfx")
                    grid = wpool.tile([128, G6], F32, tag="grid")
                    for b in range(B):
                        if "c1" in ablate:
                            nc.vector.memset(canvas[:, S[b]:S[b] + 2], 0.0)
                        else:
                            nc.gpsimd.local_scatter(
                                out_ap=canvas[:, S[b]:S[b + 1]].bitcast(I16),
                                data_ap=wtab[:, TS[b]:TS[b + 1]].bitcast(I16),
                                idxs_ap=c1x_t[:, O1[b]:O1[b + 1]],
                                channels=128,
                                num_elems=2 * W[b],
                                num_idxs=2 * NB[b],
                            )
                        if "scan" in ablate:
                            nc.vector.memset(pfx[:, S[b]:S[b] + 2], 0.0)
                        else:
                            nc.vector.tensor_tensor_scan(
                                out=pfx[:, S[b]:S[b + 1]],
                                data0=canvas[:, S[b]:S[b + 1]],
                                data1=canvas[:, S[b]:S[b + 1]],
                                initial=0.0,
                                op0=mybir.AluOpType.add,
                                op1=mybir.AluOpType.bypass)
                    for b in range(B):
                        if "c3" in ablate:
                            nc.vector.memset(grid[:, b * GSL:b * GSL + 2], 0.0)
                        else:
                            nc.gpsimd.local_scatter(
                                out_ap=grid[:, b * GSL:(b + 1) * GSL].bitcast(I16),
                                data_ap=pfx[:, S[b]:S[b + 1]].bitcast(I16),
                                idxs_ap=c3x_t[:, 2 * S[b]:2 * S[b + 1]],
                                channels=128,
                                num_elems=2 * GSL,
                                num_idxs=2 * W[b],
                            )
                    dif = wpool.tile([128, G6], F32, tag="dif")
                    nc.vector.tensor_copy(out=dif[:, 0:1], in_=grid[:, 0:1])
                    nc.vector.tensor_tensor(
                        out=dif[:, 1:G6], in0=grid[:, 1:G6],
                        in1=grid[:, 0:G6 - 1], op=mybir.AluOpType.subtract)
                    for b in range(1, B):
                        nc.vector.tensor_copy(
                            out=dif[:, b * GSL:b * GSL + 1],
                            in_=grid[:, b * GSL:b * GSL + 1])
                    acc = wpool.tile([128, GSL], F32, tag="acc")
                    nc.vector.tensor_add(out=acc[:], in0=dif[:, 0:GSL],
                                         in1=dif[:, GSL:2 * GSL])
                    for b in range(2, B):
                        nc.vector.tensor_add(
                            out=acc[:], in0=acc[:],
                            in1=dif[:, b * GSL:(b + 1) * GSL])
                    redu = wpool.tile([RR, K * GSL], F32, tag="redu")
                    for k in range(K):
                        nc.sync.dma_start(
                            out=redu[0:RR, k * GSL:(k + 1) * GSL],
                            in_=acc[16 * k:16 * (k + 1), :])
                    racc = wpool.tile([RR, GSL], F32, tag="racc")
                    nc.vector.tensor_add(out=racc[:], in0=redu[:, 0:GSL],
                                         in1=redu[:, GSL:2 * GSL])
                    for k in range(2, K):
                        nc.vector.tensor_add(
                            out=racc[:], in0=racc[:],
                            in1=redu[:, k * GSL:(k + 1) * GSL])
                    nc.vector.tensor_add(out=racc[:], in0=racc[:],
                                         in1=sown_t[:])
                    yt = wpool.tile([RR, GSL], F32, tag="yt")
                    nc.vector.tensor_scalar(
                        out=yt[:], in0=racc[:], scalar1=0.0, scalar2=None,
                        op0=mybir.AluOpType.is_gt)
                    nc.sync.dma_start(out=y.ap(), in_=yt[:])
    nc.compile()
    return nc


# ---------------------------------------------------------------- entrypoint
_NC_CACHE = {}


def kernel(x, edge_index):
    from concourse.bass_utils import run_bass_kernel_spmd
    x = np.asarray(x)
    edge_index = np.asarray(edge_index)
    geom, in_maps = prep_all(x, edge_index)
    key = (tuple(geom["W"]), tuple(geom["NB"]))
    if key not in _NC_CACHE:
        _NC_CACHE[key] = build_nc(geom, num_devices=N_CORES)
    res = run_bass_kernel_spmd(_NC_CACHE[key], in_maps,
                               core_ids=list(range(N_CORES)))
    out = np.concatenate(
        [decode_core(res.results[c]["y"]) for c in range(N_CORES)])
    return out.astype(np.int64)


# revision 12
# speedup vs baseline: 385.7895x; 1.1104x over previous
"""GNN scatter-mean via local_scatter expansion + segmented scan (trn2, 8 cores).

Per NeuronCore c (8 total): owns dsts [12500c, 12500(c+1)); edges bucketed by
dst owner (host). Only column 0 of x matters (deg>=1 => mean sign == sum sign).

Partition p = 16k + r: src chunk k = src//12500 (8 chunks), dst range r of 16
(fixed rowcounts 782*4 + 781*12, RB boundaries). Srcs within a chunk split
into B=4 bands. Stream per (p, band) = real edges (src in chunk k & band b,
dst in range r), ordered by dst slot; no dummy entries.

Device, per rep:
  C1_b  local_scatter: per-partition packed table (each needed (p,src) value
        once, mult-sorted desc; device-built prefix-dup regions serve
        occurrences t>=2) -> canvas band window, at dst-sorted positions.
  seg-scan_b (native DVE tensor_tensor_scan, state = mask*state + x; mask=0
        at run starts) -> running sums that reset per run; the value at a
        run's end IS the run's sum (no global prefix, no diff stage).
  C3_b  local_scatter: run-end values -> grid[:, b*G:(b+1)*G] dst slots;
        absent (slot, band) cells stay zero (correct contribution).
  sg += grid_b incrementally (DVE, hidden behind Pool); DMA-fold 8 chunks
  into 16 partitions; 7 adds + s_own; threshold; DMA out y [16, G].
"""
import numpy as np

import concourse.bacc as bacc
import concourse.mybir as mybir
import concourse.tile as tile

N_NODES = 100000
N_CORES = 8
NN = N_NODES // N_CORES      # 12500
K = 8                        # src chunks
RR = 16                      # dst ranges (partitions per chunk)
B = 4                        # src bands per chunk
GSL = 782                    # grid band stride (max rowcount)
GB = B * GSL
ROWCOUNTS = np.asarray([782] * 4 + [781] * 12)
RB = np.concatenate([[0], np.cumsum(ROWCOUNTS)]).astype(np.int64)
BANDQ = np.linspace(0, NN, B + 1).astype(np.int64)

F32 = mybir.dt.float32
I16 = mybir.dt.int16


# ---------------------------------------------------------------- host prep
def prep_all(x, edge_index):
    s = np.asarray(x[:, 0], np.float32)
    src = np.asarray(edge_index[0], np.int64)
    dst = np.asarray(edge_index[1], np.int64)
    owner = dst // NN
    sel_order = np.argsort(owner, kind="stable")
    bounds = np.searchsorted(owner[sel_order], np.arange(N_CORES + 1))

    cores = []
    for c in range(N_CORES):
        idx = sel_order[bounds[c]:bounds[c + 1]]
        sc = src[idx]
        dc = dst[idx] - c * NN
        k = (sc // NN).astype(np.int64)
        r = np.searchsorted(RB, dc, side="right") - 1
        p = 16 * k + r
        inchunk = sc - k * NN
        mult = np.zeros((128, NN), np.int32)
        np.add.at(mult, (p, inchunk), 1)
        slot = dc - RB[r]
        band = np.searchsorted(BANDQ, inchunk, side="right") - 1
        cores.append(dict(mult=mult, p=p, inchunk=inchunk,
                          slot=slot, band=band))

    # ---- global geometry (shared across cores: SPMD program)
    NT = []   # per band: [n_1, n_2, ...] global maxes
    W = []
    for b in range(B):
        lo, hi = int(BANDQ[b]), int(BANDQ[b + 1])
        nts = []
        t = 1
        while True:
            nt = max(int((cd["mult"][:, lo:hi] >= t).sum(1).max())
                     for cd in cores)
            if nt == 0:
                break
            nts.append(nt)
            t += 1
        NT.append(nts)
        wb = 0
        for cd in cores:
            m = cd["band"] == b
            pb, sb = cd["p"][m], cd["slot"][m]
            cnt_ps = np.zeros((128, GSL), np.int32)
            np.add.at(cnt_ps, (pb, sb), 1)
            cd[f"cnt_ps_{b}"] = cnt_ps
            wb = max(wb, int(cnt_ps.sum(1).max()))
        wb += wb % 2
        assert wb <= 1023, (b, wb)
        W.append(wb)

    NB = []   # C1 band data width (f32): n_1 + n_2 + ... (dup regions)
    DUP = []  # per band: list of (dest_off, length) prefix copies
    for b in range(B):
        nts = NT[b]
        npos = nts[0]
        dups = []
        for nt in nts[1:]:
            dups.append((npos, nt))
            npos += nt
        NB.append(npos)
        DUP.append(dups)

    S = np.concatenate([[0], np.cumsum(W)]).astype(int)
    SL = int(S[-1])
    TS = np.concatenate([[0], np.cumsum(NB)]).astype(int)
    TBW = int(TS[-1])
    O1 = np.concatenate([[0], np.cumsum([2 * n for n in NB])]).astype(int)

    geom = dict(W=[int(w) for w in W], NB=[int(n) for n in NB],
                DUP=DUP, TBW=TBW, SL=SL,
                S=[int(v) for v in S], TS=[int(v) for v in TS],
                O1=[int(v) for v in O1])

    # ---- per-core arrays
    in_maps = []
    for c, cd in enumerate(cores):
        tin = np.zeros((128, TBW), np.float32)
        c1x = np.full((128, int(O1[-1])), -1, np.int16)
        c3x = np.full((128, 2 * SL), -1, np.int16)
        mask = np.ones((128, SL), np.float32)

        # occurrence rank per (p, src) in stream order (band, slot, arrival)
        E = len(cd["p"])
        eord = np.lexsort((np.arange(E), cd["slot"], cd["band"], cd["p"]))
        pe = cd["p"][eord]; be = cd["band"][eord]; se = cd["slot"][eord]
        ice = cd["inchunk"][eord]
        pi = pe * NN + ice
        so = np.argsort(pi, kind="stable")
        cnts = np.bincount(pi, minlength=128 * NN)
        cnts = cnts[cnts > 0]
        occ_sorted = np.arange(E) - np.repeat(
            np.concatenate([[0], np.cumsum(cnts)])[:-1], cnts)
        occ = np.empty(E, np.int64)
        occ[so] = occ_sorted

        for b in range(B):
            lo, hi = int(BANDQ[b]), int(BANDQ[b + 1])
            bw = hi - lo
            mb = cd["mult"][:, lo:hi]
            order = np.argsort(-mb, axis=1, kind="stable")
            inv_order = np.empty_like(order)
            np.put_along_axis(
                inv_order, order,
                np.tile(np.arange(bw), (128, 1)), axis=1)
            chunk_base = (np.arange(128) // 16) * NN
            vals = s[(chunk_base[:, None] + lo + order)].astype(np.float32)
            ncopy = min(NB[b], bw)
            tin[:, TS[b]:TS[b] + ncopy] = vals[:, :ncopy]

            cnt_ps = cd[f"cnt_ps_{b}"]
            run_start = np.concatenate(
                [np.zeros((128, 1), np.int64),
                 np.cumsum(cnt_ps, 1)[:, :-1].astype(np.int64)], 1)
            run_end = run_start + cnt_ps - 1      # valid where cnt_ps > 0

            m = be == b
            pb, sb, icb, occb = pe[m], se[m], ice[m], occ[m]
            # rank within (p, slot) run
            key_ps = pb * GSL + sb
            so2 = np.argsort(key_ps, kind="stable")
            cnts2 = np.bincount(key_ps, minlength=128 * GSL)
            cnts2 = cnts2[cnts2 > 0]
            rank_sorted = np.arange(len(pb)) - np.repeat(
                np.concatenate([[0], np.cumsum(cnts2)])[:-1], cnts2)
            rank = np.empty(len(pb), np.int64)
            rank[so2] = rank_sorted
            pos = run_start[pb, sb] + rank          # stream pos within band

            jpos = inv_order[pb, icb - lo]
            starts = np.zeros(len(NT[b]) + 1, np.int64)
            for t in range(1, len(NT[b])):
                starts[t] = DUP[b][t - 1][0]
            tabpos = starts[occb] + jpos
            assert (tabpos < NB[b]).all()
            c1x[pb, O1[b] + 2 * tabpos] = (2 * pos).astype(np.int16)
            c1x[pb, O1[b] + 2 * tabpos + 1] = (2 * pos + 1).astype(np.int16)

            # run ends -> grid slots; mask 0 at run starts
            pres_p, pres_s = np.nonzero(cnt_ps)
            repos = run_end[pres_p, pres_s]
            c3x[pres_p, 2 * (S[b] + repos)] = (2 * pres_s).astype(np.int16)
            c3x[pres_p, 2 * (S[b] + repos) + 1] = (2 * pres_s + 1).astype(np.int16)
            mask[pres_p, S[b] + run_start[pres_p, pres_s]] = 0.0

        sown = np.zeros((RR, GSL), np.float32)
        s_own = s[c * NN:(c + 1) * NN]
        for rr in range(RR):
            n0, n1_ = int(RB[rr]), int(RB[rr + 1])
            sown[rr, :n1_ - n0] = s_own[n0:n1_]

        in_maps.append({"tin": tin, "c1x": c1x, "c3x": c3x, "mask": mask,
                        "sown": sown})
    return geom, in_maps


# ------------------------------------------------------------- numpy model
def numpy_model_core(geom, im):
    W, NB, DUP = geom["W"], geom["NB"], geom["DUP"]
    S, TS, O1 = geom["S"], geom["TS"], geom["O1"]
    SL = geom["SL"]
    wtab = im["tin"].copy()
    for b in range(B):
        for (doff, dlen) in DUP[b]:
            wtab[:, TS[b] + doff:TS[b] + doff + dlen] = \
                wtab[:, TS[b]:TS[b] + dlen]
    canvas = np.zeros((128, SL), np.float32)
    cv16 = canvas.view(np.int16)
    wt16 = wtab.view(np.int16)
    c1x = im["c1x"]
    for b in range(B):
        cv16[:, 2 * S[b]:2 * (S[b] + W[b])] = 0
        for pp in range(128):
            idx = c1x[pp, O1[b]:O1[b + 1]]
            v = idx >= 0
            cv16[pp, 2 * S[b] + idx[v].astype(np.int64)] = \
                wt16[pp, 2 * TS[b] + np.nonzero(v)[0]]
    mask = im["mask"]
    pfx = np.zeros((128, SL), np.float32)
    for b in range(B):
        state = np.zeros(128, np.float32)
        for t in range(S[b], S[b + 1]):
            state = (mask[:, t] * state + canvas[:, t]).astype(np.float32)
            pfx[:, t] = state
    grid = np.zeros((128, GB), np.float32)
    g16 = grid.view(np.int16)
    p16 = pfx.view(np.int16)
    c3x = im["c3x"]
    for b in range(B):
        g16[:, 2 * b * GSL:2 * (b + 1) * GSL] = 0
        for pp in range(128):
            idx = c3x[pp, 2 * S[b]:2 * S[b + 1]]
            v = idx >= 0
            g16[pp, 2 * b * GSL + idx[v].astype(np.int64)] = \
                p16[pp, 2 * S[b] + np.nonzero(v)[0]]
    sg = grid[:, 0:GSL].copy()
    for b in range(1, B):
        sg = sg + grid[:, b * GSL:(b + 1) * GSL]
    racc = sg[0:16] + sg[16:32]
    for k in range(2, K):
        racc = racc + sg[16 * k:16 * (k + 1), :]
    td = racc + im["sown"]
    return (td > 0).astype(np.float32)


def decode_core(yc):
    out = np.zeros(NN, np.int64)
    for rr in range(RR):
        n0, n1_ = int(RB[rr]), int(RB[rr + 1])
        out[n0:n1_] = (yc[rr, :n1_ - n0] > 0.5).astype(np.int64)
    return out


# ---------------------------------------------------------------- device IR
def build_nc(geom, num_devices=N_CORES, repeat=1, hwloop=False, ablate=()):
    W, NB, DUP = geom["W"], geom["NB"], geom["DUP"]
    S, TS, O1 = geom["S"], geom["TS"], geom["O1"]
    SL, TBW = geom["SL"], geom["TBW"]

    nc = bacc.Bacc("TRN2", target_bir_lowering=False, debug=False,
                   num_devices=num_devices)
    tin = nc.dram_tensor("tin", [128, TBW], F32, kind="ExternalInput")
    c1x = nc.dram_tensor("c1x", [128, O1[-1]], I16, kind="ExternalInput")
    c3x = nc.dram_tensor("c3x", [128, 2 * SL], I16, kind="ExternalInput")
    maskin = nc.dram_tensor("mask", [128, SL], F32, kind="ExternalInput")
    sown = nc.dram_tensor("sown", [RR, GSL], F32, kind="ExternalInput")
    y = nc.dram_tensor("y", [RR, GSL], F32, kind="ExternalOutput")

    with tile.TileContext(nc) as tc:
        with (
            tc.tile_pool(name="const", bufs=1) as cpool,
            tc.tile_pool(name="work", bufs=1) as wpool,
        ):
            wtab = cpool.tile([128, TBW], F32, tag="wtab")
            nc.sync.dma_start(out=wtab[:], in_=tin.ap())
            c1x_t = cpool.tile([128, O1[-1]], I16, tag="c1x")
            nc.sync.dma_start(out=c1x_t[:], in_=c1x.ap())
            c3x_t = cpool.tile([128, 2 * SL], I16, tag="c3x")
            nc.sync.dma_start(out=c3x_t[:], in_=c3x.ap())
            mask_t = cpool.tile([128, SL], F32, tag="mask")
            nc.sync.dma_start(out=mask_t[:], in_=maskin.ap())
            sown_t = cpool.tile([RR, GSL], F32, tag="sown")
            nc.sync.dma_start(out=sown_t[:], in_=sown.ap())

            for b in range(B):
                for (doff, dlen) in DUP[b]:
                    nc.vector.tensor_copy(
                        out=wtab[:, TS[b] + doff:TS[b] + doff + dlen],
                        in_=wtab[:, TS[b]:TS[b] + dlen])

            from contextlib import ExitStack
            with ExitStack() as stk:
                if hwloop and repeat > 1:
                    stk.enter_context(tc.For_i(0, repeat))
                    reps = (0,)
                else:
                    reps = range(repeat)
                for _rep in reps:
                    canvas = wpool.tile([128, SL], F32, tag="canvas")
                    pfx = wpool.tile([128, SL], F32, tag="pfx")
                    grid = wpool.tile([128, GB], F32, tag="grid")
                    for b in range(B):
                        if "c1" in ablate:
                            nc.vector.memset(canvas[:, S[b]:S[b] + 2], 0.0)
                        else:
                            nc.gpsimd.local_scatter(
                                out_ap=canvas[:, S[b]:S[b + 1]].bitcast(I16),
                                data_ap=wtab[:, TS[b]:TS[b + 1]].bitcast(I16),
                                idxs_ap=c1x_t[:, O1[b]:O1[b + 1]],
                                channels=128,
                                num_elems=2 * W[b],
                                num_idxs=2 * NB[b],
                            )
                        if "scan" in ablate:
                            nc.vector.memset(pfx[:, S[b]:S[b] + 2], 0.0)
                        else:
                            # segmented running sum: state = mask*state + x
                            nc.vector.tensor_tensor_scan(
                                out=pfx[:, S[b]:S[b + 1]],
                                data0=mask_t[:, S[b]:S[b + 1]],
                                data1=canvas[:, S[b]:S[b + 1]],
                                initial=0.0,
                                op0=mybir.AluOpType.mult,
                                op1=mybir.AluOpType.add)
                    sg = wpool.tile([128, GSL], F32, tag="sg")
                    for b in range(B):
                        if "c3" in ablate:
                            nc.vector.memset(grid[:, b * GSL:b * GSL + 2], 0.0)
                        else:
                            nc.gpsimd.local_scatter(
                                out_ap=grid[:, b * GSL:(b + 1) * GSL].bitcast(I16),
                                data_ap=pfx[:, S[b]:S[b + 1]].bitcast(I16),
                                idxs_ap=c3x_t[:, 2 * S[b]:2 * S[b + 1]],
                                channels=128,
                                num_elems=2 * GSL,
                                num_idxs=2 * W[b],
                            )
                        if b == 0:
                            nc.vector.tensor_copy(out=sg[:], in_=grid[:, 0:GSL])
                        else:
                            nc.vector.tensor_add(
                                out=sg[:], in0=sg[:],
                                in1=grid[:, b * GSL:(b + 1) * GSL])
                    redu = wpool.tile([RR, K * GSL], F32, tag="redu")
                    for k in range(K):
                        nc.sync.dma_start(
                            out=redu[0:RR, k * GSL:(k + 1) * GSL],
                            in_=sg[16 * k:16 * (k + 1), :])
                    racc = wpool.tile([RR, GSL], F32, tag="racc")
                    nc.vector.tensor_add(out=racc[:], in0=redu[:, 0:GSL],
                                         in1=redu[:, GSL:2 * GSL])
                    for k in range(2, K):
                        nc.vector.tensor_add(
                            out=racc[:], in0=racc[:],
                            in1=redu[:, k * GSL:(k + 1) * GSL])
                    nc.vector.tensor_add(out=racc[:], in0=racc[:],
                                         in1=sown_t[:])
                    yt = wpool.tile([RR, GSL], F32, tag="yt")
                    nc.vector.tensor_scalar(
                        out=yt[:], in0=racc[:], scalar1=0.0, scalar2=None,
                        op0=mybir.AluOpType.is_gt)
                    nc.sync.dma_start(out=y.ap(), in_=yt[:])
    nc.compile()
    return nc


# ---------------------------------------------------------------- entrypoint
_NC_CACHE = {}


def kernel(x, edge_index):
    from concourse.bass_utils import run_bass_kernel_spmd
    x = np.asarray(x)
    edge_index = np.asarray(edge_index)
    geom, in_maps = prep_all(x, edge_index)
    key = (tuple(geom["W"]), tuple(geom["NB"]))
    if key not in _NC_CACHE:
        _NC_CACHE[key] = build_nc(geom, num_devices=N_CORES)
    res = run_bass_kernel_spmd(_NC_CACHE[key], in_maps,
                               core_ids=list(range(N_CORES)))
    out = np.concatenate(
        [decode_core(res.results[c]["y"]) for c in range(N_CORES)])
    return out.astype(np.int64)
